# revision 2
# baseline (speedup 1.0000x reference)
"""Trainium2 Bass kernel for BetaBernoulliMixture — product-scan formulation.

Math (no logs, no exp):
  post[t] = 1 / (1 + k*R[t]),  k = (1-w)/w
  R[t]    = prod_{tau<t} num2[tau]/den2[tau]        (bounded: e^d, |d| < ~3)
  num2    = w1t * (obs ? a2 : b2),  den2 = w2t * (obs ? a1 : b1)
  a1 = alpha1 + s_prev (exclusive cumsum), b1 = w1t - a1, a2/b2 = a1/b1 + d*

Custom DVE ops (registered at import into concourse.dve_ops):
  BB_EXCL_SCAN: out = C0 + cumsum_excl(Src0)                      -> a1
  BB_SWSEL:     out = Src1 * sel(Src0, A, Src1 + C1 - A),
                A = C0 + cumsum_excl(Src0)                        -> num2/den2
                 (den2: Src1=w2t, C0=al1,     C1=ab1-ab2)
                 (num2: Src1=w1t, C0=al1+dal, C1=dal+dbe)
  BB_PRODSCAN:  out = C0 * cumprod(Src0 * Src1)                   -> R

Engine split per [128 x 2048] tile:
  DVE: excl-scan(a1), swsel(den2), swsel(num2), recip_approx_fast, prodscan
  ACT: w1t, w2t, a2 (f16), b2 (f16), post = table-Recip(k*R+1) (f16)
  GPS: b1 = w1t - a1 (tensor_tensor), SWDGE cast-DMAs (obs f32->f8 in,
       a1/b1 f32->f16 out)
All five outputs leave the device as fp16 (rel err ~5e-4, gate 2e-2).
"""

import numpy as np

B, T = 4096, 8192
NCORES = 8
RPC = B // NCORES        # 512 rows per core
P = 128
RC_N = RPC // P          # 4 row chunks
F = 2048
TC_N = T // F            # 4 t chunks
# al1, al1+dal, ab1-ab2, dal+dbe, dal, dbe, then per-chunk (ab1+t0, ab2+t0)
NCONST = 6 + 2 * TC_N

_PROGRAM_CACHE = {}


def _patch_act_tables():
    """Keep only reciprocal_and_small (has reciprocal + identity + copy) so
    the activation table is loaded exactly once."""
    import concourse.bacc as bacc_mod
    import concourse.hw_specs as hw_specs
    if getattr(bacc_mod, "_act_tables_patched", False):
        return
    orig = hw_specs.get_activation_tables

    def filtered(arch):
        full = orig(arch)
        return {
            name: (funcs if name == "reciprocal_and_small" else set())
            for name, funcs in full.items()
        }

    bacc_mod.get_activation_tables = filtered
    bacc_mod._act_tables_patched = True


def _register_custom_ops():
    import concourse.dve_ops as dvo
    from concourse.dve_spec import (
        Spec, Src0, Src1, C0, C1, AluOp, scan, select, lower, _has_src1,
    )
    from concourse.dve_uop import DveOpSpec

    made = {}

    def make_op(name, spec):
        if name in dvo._SUB_OPCODE_FOR_NAME:
            made[name] = next(o for o in dvo.OPS if o.name == name)
            return
        row = dvo._CUSTOM_DVE_ROW_BASE + len(dvo.OPS)
        assert row < 0x20, "out of custom-DVE opcode rows"
        dvo._SUB_OPCODE_FOR_NAME[name] = row
        shas = {}
        for ver in ("v3", "v4"):
            uops = lower(spec, ver=ver)
            s = DveOpSpec(name=name, opcode=row, uops=uops, rd1_en=_has_src1(spec))
            shas[ver] = s.sha(ver)
        op = dvo.DveOp(name=name, spec=spec, subdim=False, uops_sha=shas)
        dvo.OPS.append(op)
        dvo.CUSTOM_DVE_SPECS[name] = spec
        made[name] = op

    A = scan(AluOp.ADD, Src0, init=C0) - Src0

    def _ref_excl(in0, in1, c0, c1, c2):
        return (c0 + np.cumsum(in0, axis=-1) - in0).astype(np.float32)

    make_op("BB_EXCL_SCAN", Spec(body=A, reference=_ref_excl))

    body_swsel = Src1 * select(Src0, A, (Src1 + C1) - A)

    def _ref_swsel(in0, in1, c0, c1, c2):
        a = c0 + np.cumsum(in0, axis=-1) - in0
        return (in1 * np.where(in0 != 0, a, (in1 + c1) - a)).astype(np.float32)

    make_op("BB_SWSEL", Spec(body=body_swsel, reference=_ref_swsel))

    body_prod = scan(AluOp.MULTIPLY, Src0 * Src1, init=C0)

    def _ref_prod(in0, in1, c0, c1, c2):
        return (c0 * np.cumprod(in0.astype(np.float64) * in1, axis=-1)).astype(
            np.float32)

    make_op("BB_PRODSCAN", Spec(body=body_prod, reference=_ref_prod))

    body_b1 = (Src1 + C1) - A

    def _ref_b1(in0, in1, c0, c1, c2):
        a = c0 + np.cumsum(in0, axis=-1) - in0
        return ((in1 + c1) - a).astype(np.float32)

    make_op("BB_B1", Spec(body=body_b1, reference=_ref_b1))
    return made


def _build_program(k_mix: float):
    import concourse.bacc as bacc
    import concourse.mybir as mybir
    from concourse.tile import TileContext

    _patch_act_tables()
    ops = _register_custom_ops()

    f32 = mybir.dt.float32
    f16 = mybir.dt.float16
    Alu = mybir.AluOpType
    Act = mybir.ActivationFunctionType

    nc = bacc.Bacc()
    obs_d = nc.dram_tensor("obs", [RPC, T], f32, kind="ExternalInput")
    rcst_d = nc.dram_tensor("rowconst", [RPC, NCONST], f32, kind="ExternalInput")
    a1_o = nc.dram_tensor("a1_out", [RPC, T], f16, kind="ExternalOutput")
    b1_o = nc.dram_tensor("b1_out", [RPC, T], f16, kind="ExternalOutput")
    a2_o = nc.dram_tensor("a2_out", [RPC, T], f16, kind="ExternalOutput")
    b2_o = nc.dram_tensor("b2_out", [RPC, T], f16, kind="ExternalOutput")
    pm_o = nc.dram_tensor("post_out", [RPC, T], f16, kind="ExternalOutput")

    V, G, S = None, None, None  # set below for brevity

    with TileContext(nc) as tc:
        V, G, S = nc.vector, nc.gpsimd, nc.scalar
        with (
            tc.tile_pool(name="consts", bufs=1) as cpool,
            tc.tile_pool(name="rows", bufs=2) as rpool,
            tc.tile_pool(name="work", bufs=2) as wpool,
            tc.tile_pool(name="chain", bufs=2) as hpool,
        ):
            iota_t = cpool.tile([P, F], f32, tag="iota")
            G.iota(iota_t[:], pattern=[[1, F]], base=0, channel_multiplier=0,
                   allow_small_or_imprecise_dtypes=True)

            for rc in range(RC_N):
                r0 = rc * P
                rows_t = rpool.tile([P, NCONST], f32, tag="rows")
                nc.sync.dma_start(rows_t[:], rcst_d[r0:r0 + P, :])
                al1 = rows_t[:, 0:1]
                a2i = rows_t[:, 1:2]
                c1d = rows_t[:, 2:3]
                c1n = rows_t[:, 3:4]
                dal = rows_t[:, 4:5]
                dbe = rows_t[:, 5:6]

                c0d = al1
                c0n = a2i
                prev_R = None
                for tci in range(TC_N):
                    t0 = tci * F
                    ab1t = rows_t[:, 6 + 2 * tci:7 + 2 * tci]
                    ab2t = rows_t[:, 7 + 2 * tci:8 + 2 * tci]

                    obs_t = wpool.tile([P, F], f32, tag="obs")
                    nc.sync.dma_start(obs_t[:], obs_d[r0:r0 + P, t0:t0 + F])

                    w1_t = wpool.tile([P, F], f32, tag="w1")
                    w2_t = wpool.tile([P, F], f32, tag="w2")
                    S.activation(w1_t[:], iota_t[:], Act.Identity, bias=ab1t)
                    S.activation(w2_t[:], iota_t[:], Act.Identity, bias=ab2t)

                    a1_t = wpool.tile([P, F], f32, tag="a1")
                    V._custom_dve(ops["BB_EXCL_SCAN"], out=a1_t[:],
                                  in0=obs_t[:], s0=c0d)
                    den2_t = wpool.tile([P, F], f32, tag="den2")
                    V._custom_dve(ops["BB_SWSEL"], out=den2_t[:],
                                  in0=obs_t[:], in1=w2_t[:], s0=c0d, s1=c1d)
                    num2_t = wpool.tile([P, F], f32, tag="num2")
                    V._custom_dve(ops["BB_SWSEL"], out=num2_t[:],
                                  in0=obs_t[:], in1=w1_t[:], s0=c0n, s1=c1n)
                    rden2_t = wpool.tile([P, F], f32, tag="rden2")
                    # approx-fast recip with the final NR constant bumped to
                    # centre its one-sided error (else the bias compounds
                    # ~1e-2 over the 8192-step product)
                    from concourse.dve_ops import RECIPROCAL_APPROX_FAST
                    V._custom_dve(RECIPROCAL_APPROX_FAST, out=rden2_t[:],
                                  in0=den2_t[:], s0=-0.23549792, s1=2.0017324,
                                  imm2=2.0 + 1.474e-6)

                    R_t = wpool.tile([P, F + 1], f32, tag="R")
                    if tci == 0:
                        V.memset(R_t[:, 0:1], 1.0)
                        r_init = 1.0
                    else:
                        r_init = prev_R[:, F:F + 1]
                        V.tensor_copy(R_t[:, 0:1], r_init)
                    V._custom_dve(ops["BB_PRODSCAN"], out=R_t[:, 1:F + 1],
                                  in0=num2_t[:], in1=rden2_t[:], s0=r_init)

                    b1_t = wpool.tile([P, F], f32, tag="b1")
                    # b1 on DVE: a GPS tensor_tensor co-streaming w1/a1 with
                    # DVE or ACT costs those engines ~2-3x on shared tiles
                    V._custom_dve(ops["BB_B1"], out=b1_t[:], in0=obs_t[:],
                                  in1=w1_t[:], s0=c0d, s1=0.0)

                    a2_t = wpool.tile([P, F], f16, tag="a2")
                    b2_t = wpool.tile([P, F], f16, tag="b2")
                    S.activation(a2_t[:], a1_t[:], Act.Identity, bias=dal)
                    S.activation(b2_t[:], b1_t[:], Act.Identity, bias=dbe)

                    post_t = wpool.tile([P, F], f16, tag="post")
                    ins_l = [S.lower_ap(R_t[:, 0:F])]
                    for v in (1.0, k_mix, 0.0):  # bias=1, scale=k, alpha
                        ins_l.append(mybir.ImmediateValue(
                            dtype=mybir.dt.float32, value=v))
                    S.add_instruction(mybir.InstActivation(
                        name=S.bass.get_next_instruction_name(),
                        func=Act.Reciprocal, ins=ins_l,
                        outs=[S.lower_ap(post_t[:])]))

                    # outputs: a1/b1 via SWDGE cast f32->f16; rest direct f16
                    G.dma_start(a1_o[r0:r0 + P, t0:t0 + F], a1_t[:])
                    G.dma_start(b1_o[r0:r0 + P, t0:t0 + F], b1_t[:])
                    nc.sync.dma_start(a2_o[r0:r0 + P, t0:t0 + F], a2_t[:])
                    nc.sync.dma_start(b2_o[r0:r0 + P, t0:t0 + F], b2_t[:])
                    nc.sync.dma_start(pm_o[r0:r0 + P, t0:t0 + F], post_t[:])

                    if tci + 1 < TC_N:
                        # chain: c0d' = a1[F-1] + obs[F-1]; c0n' = c0d' + dal
                        c0d_n = hpool.tile([P, 1], f32, tag="c0d")
                        V.tensor_tensor(c0d_n[:], a1_t[:, F - 1:F],
                                        obs_t[:, F - 1:F], Alu.add)
                        c0n_n = hpool.tile([P, 1], f32, tag="c0n")
                        V.tensor_tensor(c0n_n[:], c0d_n[:], dal, Alu.add)
                        c0d, c0n = c0d_n[:], c0n_n[:]
                    prev_R = R_t
    nc.finalize()
    return nc


def _pack_rowconst(alpha1, beta1, alpha2, beta2):
    a1 = np.asarray(alpha1, np.float32)
    b1 = np.asarray(beta1, np.float32)
    a2 = np.asarray(alpha2, np.float32)
    b2 = np.asarray(beta2, np.float32)
    dal = a2 - a1
    dbe = b2 - b1
    ab1 = a1 + b1
    ab2 = a2 + b2
    cols = [a1, a1 + dal, ab1 - ab2, dal + dbe, dal, dbe]
    for tci in range(TC_N):
        t0 = np.float32(tci * F)
        cols.append(ab1 + t0)
        cols.append(ab2 + t0)
    return np.ascontiguousarray(np.stack(cols, axis=1), dtype=np.float32)


def kernel(obs_seq, alpha1, beta1, alpha2, beta2, mixweight):
    from concourse.bass_utils import run_bass_kernel_spmd

    w = float(np.float32(mixweight))
    k_mix = float(np.float32((1.0 - w) / w))
    if k_mix not in _PROGRAM_CACHE:
        _PROGRAM_CACHE[k_mix] = _build_program(k_mix)
    nc = _PROGRAM_CACHE[k_mix]

    obs_seq = np.ascontiguousarray(obs_seq, dtype=np.float32)
    rowconst = _pack_rowconst(alpha1, beta1, alpha2, beta2)
    in_maps = []
    for c in range(NCORES):
        r0 = c * RPC
        in_maps.append({
            "obs": obs_seq[r0:r0 + RPC],
            "rowconst": rowconst[r0:r0 + RPC],
        })
    res = run_bass_kernel_spmd(nc, in_maps, core_ids=list(range(NCORES)))
    out = np.empty((5, B, T), np.float32)
    names = ["a1_out", "b1_out", "a2_out", "b2_out", "post_out"]
    for c in range(NCORES):
        r0 = c * RPC
        for kk, name in enumerate(names):
            out[kk, r0:r0 + RPC] = np.asarray(
                res.results[c][name]).astype(np.float32)
    return out
